# revision 35
# baseline (speedup 1.0000x reference)
"""Discriminative-loss kernel for Trainium2 (Bass/Tile), 8-core data-parallel.

Layout / algorithm (per core = one batch sample, SPMD over 8 cores):
  All layout work (transpose, bf16 cast, one-hot expansion) happens on the
  HOST; the device does only matmuls + elementwise passes.

  inputs per core (all prepped host-side):
    xt    [128, NB=2048, 17] bf16  point-major: xt[p,b,dd] = x[dd, 128b+p],
                                   col 16 = 1.0 (counts come free)
    oh_t  [128, NB, 8]  bf16  onehot, same point-major layout
    xbf   [128, M=32768] bf16  d-major: row 16j+dd, col c -> x[dd, j*M+c]
    oh_jk [64, M]  bf16  oh_jk[8j+k, c] = (lab[j*M+c]==k)
    lab_s [128, 2048] bf16  stripe-layout labels (matches e2, see below)
    red8  [128, 32] bf16  reduce-over-dd stationary (cols 8..31 zero)

  pass 1:  S-matmuls: stationary = 16 blocks of onehot [128, 128],
           moving = 16 blocks of xt [128, 272], accumulate PSUM [128, 272];
           fold 16 diagonal slabs -> [8, 17] = cluster sums | counts.
  centers: reciprocal counts, c_bf [8,16] bf16, broadcast to blockdiag
           w_cblk [64, 128].
  pass 2:  per 2048-col tile t: c_own matmul (w_cblk x oh_jk) -> PSUM,
           DVE subtract (x - c_own, PSUM operand), ACT square, red8 matmul
           reduce over dd -> s^2 [32v+j, f], ACT sqrt -> s bf16, stripe DMA
           -> e2 group tile [128, 512]:
             e2[32v + 4j + q, 128*(t%4) + c] = s of point
                 j*M + 2048t + 512v + 128q + c
  epilogue (per 4-tile group, overlapped with pass 2): ACT relu(e-1),
           ACT square, 8x DVE masked accumulate -> V [128, 8, 4].
  host:    centers/dist/reg terms + final mean from [8,17] sums|counts and
           V partials (O(K^2 d) flops on reduced stats only).
"""

import contextlib
import ctypes
import sys
import types

import numpy as np

# ---------------------------------------------------------------------------
# problem constants (hardcoded per contract)
B, D, HH, WW, K = 8, 16, 512, 512, 8
N = HH * WW            # 262144 points per sample
J = 8                  # chunks (e-layout granularity)
NCORES = 8
DELTA_VAR = 1.0
DELTA_DIST = 2.0

_BF16 = None  # ml_dtypes.bfloat16, resolved lazily


def _bf16():
    global _BF16
    if _BF16 is None:
        import ml_dtypes

        _BF16 = np.dtype(ml_dtypes.bfloat16)
    return _BF16


# ---------------------------------------------------------------------------
# walrus workaround: this toolchain allows only ONE sync-wait per
# instruction; spread extras onto preceding same-engine nops.
def _split_multi_waits(nc):
    from concourse import mybir

    n = 0
    for f in nc.m.functions:
        for bb in f.blocks:
            new_insts = []
            for ins in bb.instructions:
                si = getattr(ins, "sync_info", None)
                waits = list(si.on_wait) if si is not None and si.on_wait else []
                if len(waits) > 1:
                    for w in waits[:-1]:
                        n += 1
                        new_insts.append(
                            mybir.InstNoOp(
                                name=f"I-waitsplit-{n}",
                                engine=ins.engine,
                                bass_nofuse=True,
                                sync_info=mybir.SyncInfo(on_wait=[w], on_update=[]),
                            )
                        )
                    si.on_wait = waits[-1:]
                new_insts.append(ins)
            bb.instructions[:] = new_insts


# ---------------------------------------------------------------------------
# NTFF profiling hook (axon): lets run_bass_kernel_spmd(trace=True) work in
# this container. Harmless if the .so lacks the symbols.
def install_ntff_hook():
    try:
        import antenv

        if "antenv.axon_hooks" in sys.modules:
            return
        so_path = "/opt/axon/libaxon_pjrt.so"
        lib = ctypes.CDLL(so_path)
        if not hasattr(lib, "axon_start_nrt_profile"):
            return
        lib.axon_start_nrt_profile.argtypes = [
            ctypes.POINTER(ctypes.c_int64),
            ctypes.c_size_t,
        ]
        lib.axon_start_nrt_profile.restype = ctypes.c_int64
        lib.axon_stop_nrt_profile.argtypes = [ctypes.c_char_p]
        lib.axon_stop_nrt_profile.restype = ctypes.c_int64

        @contextlib.contextmanager
        def _hook(output_dir, device_ids):
            import jax

            jax.devices()
            if device_ids:
                ids = (ctypes.c_int64 * len(device_ids))(*device_ids)
                rc = lib.axon_start_nrt_profile(ids, len(device_ids))
            else:
                rc = lib.axon_start_nrt_profile(None, 0)
            if rc != 0:
                raise RuntimeError(f"axon_start_nrt_profile rc={rc}")
            try:
                yield
            finally:
                n = lib.axon_stop_nrt_profile(str(output_dir).encode())
                print(f"ntff profile: {n} file(s) -> {output_dir}", file=sys.stderr)

        mod = types.ModuleType("antenv.axon_hooks")
        mod.get_axon_ntff_profile_hook = lambda: _hook
        mod.set_axon_ntff_profile_hook = lambda h: None
        sys.modules["antenv.axon_hooks"] = mod
        antenv.axon_hooks = mod
    except Exception:
        pass


# ---------------------------------------------------------------------------
def build_nc(nt=16, num_devices=NCORES):
    """Build the Bass program.  nt = number of 2048-wide column tiles of the
    per-core d-major layout (16 for the full problem; smaller for sim;
    must be a multiple of 4 for the epilogue grouping).
    """
    import concourse.bass as bass
    import concourse.tile as tile
    from concourse import mybir

    assert nt == 16 or (1 <= nt < 16 and nt % 4 == 0) or nt in (1, 2)
    M = 2048 * nt          # points per chunk
    NPTS = J * M           # points per core
    NB = NPTS // 128       # 128-point blocks (2048 for full problem)
    SLABS = 16 if nt >= 4 else nt * 4
    BPS = NB // SLABS      # blocks per slab
    GPS = BPS // 16        # G=16-block matmul groups per slab
    if nt >= 4:
        GRPS = [4] * (nt // 4)   # tiles per epilogue group
    else:
        GRPS = [nt]
    NGRP = len(GRPS)

    fp32 = mybir.dt.float32
    bf16 = mybir.dt.bfloat16
    fp8 = mybir.dt.float8e4

    nc = bass.Bass(
        "TRN2", target_bir_lowering=False, debug=False, num_devices=num_devices
    )

    xt_in = nc.dram_tensor("xt", [128, NB, 17], fp8, kind="ExternalInput").ap()
    oht_in = nc.dram_tensor("oh_t", [128, NB, K], fp8, kind="ExternalInput").ap()
    xbf_in = nc.dram_tensor("xbf", [128, M], bf16, kind="ExternalInput").ap()
    oh_jk = nc.dram_tensor("oh_jk", [64, M], fp8, kind="ExternalInput").ap()
    lab_s = nc.dram_tensor(
        "lab_s", [128, 512 * NGRP], bf16, kind="ExternalInput"
    ).ap()  # banded layout, see prep_core_inputs
    # [128, 4, 32] j-selection stationaries: red4[p, m, c] = 1 iff
    # c == p//16 + 8m.  Variant m routes a tile's s^2 rows to band 8m of
    # each 32-partition group; other cols are zero so the matmul writes
    # (accumulates) zeros there.
    red4 = nc.dram_tensor("red4", [128, 4, 32], bf16, kind="ExternalInput").ap()
    # redq[tt][64h + 8j + k, 32h + 8tt + j] = -2:  contracts qm rows over k
    # and drops the result (times -2) into the same ps_g band as red4.
    redq = nc.dram_tensor("redq", [128, 4, 64], bf16, kind="ExternalInput").ap()
    id8 = nc.dram_tensor("id8", [K, K], bf16, kind="ExternalInput").ap()
    out_stats = nc.dram_tensor(
        "out_stats", [K, D + 1], fp32, kind="ExternalOutput"
    ).ap()
    out_var = nc.dram_tensor(
        "out_var", [128, K * NGRP], fp32, kind="ExternalOutput"
    ).ap()

    with tile.TileContext(nc) as tc, contextlib.ExitStack() as ctx:
        # ---------------- pools
        xbf_pool = ctx.enter_context(tc.tile_pool(name="xbf", bufs=nt))
        xt_pool = ctx.enter_context(tc.tile_pool(name="xt", bufs=3))
        oht_pool = ctx.enter_context(tc.tile_pool(name="oht", bufs=3))
        singles = ctx.enter_context(tc.tile_pool(name="singles", bufs=1))
        tiny = ctx.enter_context(tc.tile_pool(name="tiny", bufs=1))
        p2 = ctx.enter_context(tc.tile_pool(name="p2", bufs=3))
        p2b = ctx.enter_context(tc.tile_pool(name="p2b", bufs=2))
        epi = ctx.enter_context(tc.tile_pool(name="epi", bufs=2))
        ps_s_pool = ctx.enter_context(
            tc.tile_pool(name="ps_s", bufs=1, space="PSUM")
        )
        psq_pool = ctx.enter_context(tc.tile_pool(name="psq", bufs=2, space="PSUM"))
        ps_g_pool = ctx.enter_context(
            tc.tile_pool(name="ps_g", bufs=1, space="PSUM")
        )

        # ---------------- pass 1: cluster sums from point-major layout
        ps_s = ps_s_pool.tile([128, 272], fp32)
        ps_g_tiles = []
        for g in range(NGRP):
            pgt = ps_g_pool.tile([128, 512], fp32, tag=f"g{g}", name=f"ps_g{g}")
            ps_g_tiles.append(pgt)
        xbf = []
        cnt = 0
        nmm = SLABS * GPS
        # small constants (needed by the hoisted n2 path)
        red4_sb = singles.tile([128, 4, 32], bf16)
        nc.sync.dma_start(out=red4_sb[:], in_=red4)
        redq_sb = singles.tile([128, 4, 64], bf16)
        nc.scalar.dma_start(out=redq_sb[:], in_=redq)
        id8_sb = singles.tile([K, K], bf16)
        nc.scalar.dma_start(out=id8_sb[:], in_=id8)
        lab_s_sb = singles.tile([128, 512 * NGRP], bf16)
        nc.scalar.dma_start(out=lab_s_sb[:], in_=lab_s)

        ohjk_all = {}
        for s in range(SLABS):
            xt_sb = xt_pool.tile([128, BPS, 17], fp8, tag="xt")
            nc.sync.dma_start(out=xt_sb[:], in_=xt_in[:, BPS * s : BPS * (s + 1), :])
            oh_sb = oht_pool.tile([128, BPS, K], fp8, tag="oht")
            nc.scalar.dma_start(
                out=oh_sb[:], in_=oht_in[:, BPS * s : BPS * (s + 1), :]
            )
            # stream d-major x for pass 2 in the second half of pass 1
            # (earlier starves the xt/oh stream; later starves sq_x hoisting)
            for t in (
                range(
                    nt * (s - SLABS // 2) // (SLABS // 2),
                    nt * (s - SLABS // 2 + 1) // (SLABS // 2),
                )
                if s >= SLABS // 2
                else []
            ):
                xb = xbf_pool.tile([128, 2048], bf16, tag="xbf")
                nc.scalar.dma_start(
                    out=xb[:], in_=xbf_in[:, 2048 * t : 2048 * (t + 1)]
                )
                xbf.append(xb)
                # n2 path (no center dependence): sq_x and its band-reduce
                # run here, filling otherwise idle ACT/PE time.
                grp, tt = t // 4, t % 4
                if grp < NGRP:
                    sqx = p2.tile([128, 2048], bf16, tag="sqx")
                    nc.scalar.square(sqx[:], xb[:])
                    pg = ps_g_tiles[grp]
                    for v in range(4):
                        nc.tensor.matmul(
                            pg[32 * v : 32 * v + 32, :],
                            red4_sb[:, tt, :],
                            sqx[:, 512 * v : 512 * (v + 1)],
                            start=(tt == 0),
                            stop=False,
                            tile_position=(0, 32 * v),
                            skip_group_check=True,
                        )
            # prefetch the first oh_jk tiles so pass 2 starts immediately
            if s >= SLABS - 3 and nt >= 4:
                t = s - (SLABS - 3)
                oj = p2.tile([128, 2048], fp8, tag="ohjk")
                nc.sync.dma_start(
                    out=oj[0:64, :], in_=oh_jk[:, 2048 * t : 2048 * (t + 1)]
                )
                nc.scalar.dma_start(
                    out=oj[64:128, 0:1536],
                    in_=oh_jk[:, 2048 * t + 512 : 2048 * (t + 1)],
                )
                ohjk_all[t] = oj
            for g in range(GPS):
                nc.tensor.matmul(
                    ps_s[:],
                    oh_sb[:, 16 * g : 16 * (g + 1), :],
                    xt_sb[:, 16 * g : 16 * (g + 1), :],
                    start=(cnt == 0),
                    stop=(cnt == nmm - 1),
                )
                cnt += 1


        # ---------------- fold diagonal slabs -> [8, 17] sums|counts
        # (PSUM engine reads must be 32-partition aligned: copy to SBUF
        # first; DMA cannot read PSUM)
        s128 = tiny.tile([128, 272], fp32, tag="s128")
        nc.vector.tensor_scalar(
            out=s128[:],
            in0=ps_s[:],
            scalar1=0.0,
            scalar2=None,
            op0=mybir.AluOpType.add,
        )
        fold = tiny.tile([K, 16, D + 1], fp32, tag="fold")
        for g in range(16):
            eng = (nc.sync, nc.scalar)[g % 2]
            eng.dma_start(
                out=fold[:, g, :],
                in_=s128[8 * g : 8 * (g + 1), 17 * g : 17 * (g + 1)],
            )
        f2 = tiny.tile([K, 8, D + 1], fp32, tag="f2")
        nc.vector.tensor_add(f2[:], fold[:, 0:8, :], fold[:, 8:16, :])
        f3 = tiny.tile([K, 4, D + 1], fp32, tag="f3")
        nc.vector.tensor_add(f3[:], f2[:, 0:4, :], f2[:, 4:8, :])
        f4 = tiny.tile([K, 2, D + 1], fp32, tag="f4")
        nc.vector.tensor_add(f4[:], f3[:, 0:2, :], f3[:, 2:4, :])
        s_sb = tiny.tile([K, D + 1], fp32, tag="s_sb")
        nc.vector.tensor_add(s_sb[:], f4[:, 0, :], f4[:, 1, :])
        nc.sync.dma_start(out=out_stats, in_=s_sb[:])

        # ---------------- centers
        rec = tiny.tile([K, 1], fp32, tag="rec")
        nc.vector.reciprocal(rec[:], s_sb[:, D : D + 1])
        c_bf = tiny.tile([K, D], bf16, tag="c_bf")
        nc.vector.tensor_scalar(
            out=c_bf[:],
            in0=s_sb[:, 0:D],
            scalar1=rec[:],
            scalar2=None,
            op0=mybir.AluOpType.mult,
        )
        # w2[(16j+dd), (8j'+k)] = delta(j,j') * c[k, dd]  (Q = x . c_k)
        # (transpose c_bf via the PE: out = in^T @ I)
        ps_t = ps_s_pool.tile([D, K], bf16)
        nc.tensor.matmul(
            ps_t[:], c_bf[:], id8_sb[:], is_transpose=True, start=True, stop=True
        )
        c_t = tiny.tile([D, K], bf16, tag="c_t")
        nc.vector.tensor_scalar(
            out=c_t[:],
            in0=ps_t[:],
            scalar1=0.0,
            scalar2=None,
            op0=mybir.AluOpType.add,
        )
        w2 = singles.tile([128, 64], bf16)
        nc.vector.memset(w2[:], 0.0)
        for j in range(J):
            eng = (nc.sync, nc.scalar)[j % 2]
            eng.dma_start(
                out=w2[16 * j : 16 * j + D, 8 * j : 8 * j + K], in_=c_t[:]
            )
        # c2h[(64h + 8j + k), 0] = ||c_k||^2 / 2
        cc = tiny.tile([K, D], fp32, tag="cc")
        c2 = tiny.tile([K, 1], fp32, tag="c2")
        nc.vector.tensor_tensor(
            out=cc[:], in0=c_bf[:], in1=c_bf[:], op=mybir.AluOpType.mult
        )
        cch = tiny.tile([K, D], fp32, tag="cch")
        nc.vector.tensor_scalar(
            out=cch[:],
            in0=cc[:],
            scalar1=0.5,
            scalar2=0.0,
            op0=mybir.AluOpType.mult,
            op1=mybir.AluOpType.add,
            accum_out=c2[:],
        )
        c2h = tiny.tile([128, 1], fp32, tag="c2h")
        for jh in range(16):
            eng = (nc.sync, nc.scalar)[jh % 2]
            eng.dma_start(out=c2h[8 * jh : 8 * jh + K, :], in_=c2[:])

        neg1 = tiny.tile([128, 1], fp32, tag="neg1")
        nc.vector.memset(neg1[:], -float(DELTA_VAR))
        v_sb = tiny.tile([128, K, NGRP], fp32, tag="v_sb")

        # ---------------- pass 2
        redk_pend = []

        def _flush_redk(n):
            while redk_pend and (n < 0 or len(redk_pend) > n or n == 0):
                pg_, off_, red_, qm_, stop_ = redk_pend.pop(0)
                nc.tensor.matmul(
                    pg_[off_ : off_ + 64, :],
                    red_,
                    qm_[:],
                    start=False,
                    stop=stop_,
                    tile_position=(0, off_),
                    skip_group_check=True,
                )
                if n == 0 and not redk_pend:
                    break
                if 0 <= n >= len(redk_pend):
                    break

        tbase = 0
        for grp in range(NGRP):
            TPG = GRPS[grp]
            # ps_g[32v + 8tt + j, f] already holds n2 = sum_dd x^2 (from the
            # hoisted pass); now accumulate -2*(Q_own - c2own/2) onto it.
            ps_g = ps_g_tiles[grp]
            for tt in range(TPG):
                t = tbase + tt
                if t in ohjk_all:
                    ohjk_sb = ohjk_all.pop(t)
                else:
                    ohjk_sb = p2.tile([128, 2048], fp8, tag="ohjk")
                    nc.sync.dma_start(
                        out=ohjk_sb[0:64, :],
                        in_=oh_jk[:, 2048 * t : 2048 * (t + 1)],
                    )
                    nc.scalar.dma_start(
                        out=ohjk_sb[64:128, 0:1536],
                        in_=oh_jk[:, 2048 * t + 512 : 2048 * (t + 1)],
                    )
                for pp in range(2):
                    # col-tiled pair: slab 2pp -> out rows 0:64,
                    #                 slab 2pp+1 -> out rows 64:128
                    psq = psq_pool.tile([128, 512], fp32)
                    nc.tensor.matmul(
                        psq[0:64, :],
                        w2[:],
                        xbf[t][:, 1024 * pp : 1024 * pp + 512],
                        start=True,
                        stop=True,
                        tile_position=(0, 0),
                        skip_group_check=True,
                    )
                    nc.tensor.matmul(
                        psq[64:128, :],
                        w2[:],
                        xbf[t][:, 1024 * pp + 512 : 1024 * pp + 1024],
                        start=True,
                        stop=True,
                        tile_position=(0, 64),
                        skip_group_check=True,
                    )
                    # qm = (Q - c2/2) * onehot ; ohjk rows 64:128 are the
                    # +512-shifted copy so one AP covers both slabs.
                    qm = p2b.tile([128, 512], bf16, tag="qm")
                    nc.vector.scalar_tensor_tensor(
                        out=qm[:],
                        in0=psq[:],
                        scalar=c2h[:],
                        in1=ohjk_sb[:, 1024 * pp : 1024 * pp + 512],
                        op0=mybir.AluOpType.subtract,
                        op1=mybir.AluOpType.mult,
                    )
                    # defer the redk matmul by 2 pairs: an immediately-issued
                    # redk waits on its stt in the PE FIFO and blocks the
                    # next pair's mmQs (head-of-line).
                    redk_pend.append(
                        (ps_g, 64 * pp, redq_sb[:, tt, :], qm,
                         tt == TPG - 1)
                    )
                    if len(redk_pend) > 2:
                        _flush_redk(1)
            _flush_redk(0)
            # clamp tiny negative s^2 (expansion rounding) before sqrt
            sg0 = p2b.tile([128, 512], bf16, tag="sg0")
            nc.scalar.activation(
                out=sg0[:],
                in_=ps_g[:],
                func=mybir.ActivationFunctionType.Relu,
            )
            s_g = p2b.tile([128, 512], bf16, tag="s_g")
            nc.scalar.sqrt(s_g[:], sg0[:])
            # epilogue for this group's 65536 points (overlaps next group)
            m_e = epi.tile([128, 512], bf16, tag="m_e")
            nc.scalar.activation(
                out=m_e[:],
                in_=s_g[:],
                func=mybir.ActivationFunctionType.Relu,
                bias=neg1[:],
                scale=1.0,
            )
            msq = epi.tile([128, 512], bf16, tag="msq")
            nc.scalar.square(msq[:], m_e[:])
            scr = epi.tile([128, 512], bf16, tag="scr")
            for k in range(K):
                nc.vector.scalar_tensor_tensor(
                    out=scr[:],
                    in0=lab_s_sb[:, 512 * grp : 512 * (grp + 1)],
                    scalar=float(k),
                    in1=msq[:],
                    op0=mybir.AluOpType.is_equal,
                    op1=mybir.AluOpType.mult,
                    accum_out=v_sb[:, k, grp : grp + 1],
                )
            nc.sync.dma_start(
                out=out_var[:, K * grp : K * (grp + 1)],
                in_=v_sb[:, :, grp],
            )
            tbase += TPG

    _split_multi_waits(nc)
    return nc


# ---------------------------------------------------------------------------
# host-side input prep
def prep_core_inputs(x_c, labels_c, nt=16):
    """x_c fp32 [16, NPTS] (contiguous), labels_c int [NPTS] -> in_map."""
    M = 2048 * nt
    NPTS = J * M
    NB = NPTS // 128
    if nt >= 4:
        GRPS = [4] * (nt // 4)
    else:
        GRPS = [nt]
    NGRP = len(GRPS)
    bf = _bf16()
    lab = labels_c.astype(np.int64)

    import ml_dtypes as _mld

    _f8 = np.dtype(_mld.float8_e4m3fn)
    # xt: [128, NB, 17] point-major, col 16 = ones (fp8: feeds DoubleRow)
    xt = np.empty((128, NB, D + 1), dtype=_f8)
    xt[:, :, :D] = x_c.reshape(D, NB, 128).transpose(2, 1, 0).astype(_f8)
    xt[:, :, D] = 1.0

    import ml_dtypes

    f8 = np.dtype(ml_dtypes.float8_e4m3fn)
    # oh_t: [128, NB, K] point-major onehot (fp8: 0/1 exact)
    oh_t = (
        lab.reshape(NB, 128).T[:, :, None] == np.arange(K)[None, None, :]
    ).astype(f8)

    # xbf: [128, M] d-major (row 16j+dd)
    xbf = np.ascontiguousarray(
        x_c.reshape(D, J, M).transpose(1, 0, 2).reshape(128, M)
    ).astype(bf)

    # oh_jk[8*j + k, c] = (lab[j*M + c] == k)
    oh_jk = (
        (lab.reshape(J, 1, M) == np.arange(K).reshape(1, K, 1))
        .reshape(64, M)
        .astype(f8)
    )

    # lab_s banded layout: lab_s[32v + 8tt + j, 512*grp + f] =
    #   lab[j*M + 2048*(tbase_grp + tt) + 512v + f]   (tt < GRPS[grp])
    l5 = lab.reshape(J, nt, 4, 512)        # j, t, v, f
    lab_s = np.full((128, 512 * NGRP), -1.0, dtype=np.float32)
    tbase = 0
    for grp, tpg in enumerate(GRPS):
        for tt in range(tpg):
            for v in range(4):
                rows = 32 * v + 8 * tt + np.arange(J)
                lab_s[rows, 512 * grp : 512 * (grp + 1)] = l5[:, tbase + tt, v, :]
        tbase += tpg
    lab_s = lab_s.astype(bf)

    red4 = np.zeros((128, 4, 32), dtype=bf)
    for p in range(128):
        for m in range(4):
            red4[p, m, p // 16 + 8 * m] = 1.0

    id8 = np.eye(K, dtype=bf)

    # redq[64h + 8j + k, tt, 32h + 8tt + j] = -2
    redq = np.zeros((128, 4, 64), dtype=bf)
    for h in range(2):
        for j in range(J):
            for k in range(K):
                for tt in range(4):
                    redq[64 * h + 8 * j + k, tt, 32 * h + 8 * tt + j] = -2.0

    return {
        "xt": xt,
        "oh_t": oh_t,
        "xbf": xbf,
        "oh_jk": oh_jk,
        "lab_s": lab_s,
        "red4": red4,
        "redq": redq,
        "id8": id8,
    }


def finish_host(stats_list, var_list):
    """Combine per-core [K, D+1] sums|counts and [128, K*NGRP] var partials."""
    losses = []
    for stats, vparts in zip(stats_list, var_list):
        S = stats[:, :D].astype(np.float64)
        m = stats[:, D].astype(np.float64)
        centers = S / m[:, None]
        V = vparts.astype(np.float64).reshape(128, K, -1).sum(axis=(0, 2))  # [K]
        var_term = np.mean(V / m)
        dif = centers[None, :, :] - centers[:, None, :]
        dmat = np.sqrt((dif**2).sum(-1))
        dmat = dmat + np.eye(K) * DELTA_DIST
        dist_cost = np.clip(DELTA_DIST - dmat, 0.0, None) ** 2
        dist_term = dist_cost.sum() / (K * (K - 1))
        cn = np.sqrt((centers**2).sum(-1))
        reg_term = np.mean(np.clip(cn - np.sqrt(float(D)), 0.0, None) ** 2)
        losses.append(var_term + dist_term + reg_term)
    return np.float32(np.mean(losses))


# ---------------------------------------------------------------------------
_CACHE = {}


def _get_nc():
    if "nc" not in _CACHE:
        _CACHE["nc"] = build_nc(nt=16, num_devices=NCORES)
    return _CACHE["nc"]


_LDW_PATCHED = False


def _enable_ldw_opt():
    """Rewrite --enable-ldw-opt=false -> true in the walrus invocation so
    back-to-back matmuls sharing a stationary skip redundant LDWEIGHTS."""
    global _LDW_PATCHED
    if _LDW_PATCHED:
        return
    import concourse.bass_utils as bu

    orig = bu.run_command

    # NOTE: walrus rejects --enable-ldw-opt=true for this codegen path
    # (InstLdweights "not compatible with LDW optimization"); keep default.
    _ = orig
    _LDW_PATCHED = True


def run_device(in_maps, trace=False):
    from concourse.bass_utils import run_bass_kernel_spmd

    _enable_ldw_opt()
    if trace:
        install_ntff_hook()
    nc = _get_nc()
    return run_bass_kernel_spmd(
        nc, in_maps, core_ids=list(range(NCORES)), trace=trace
    )


def kernel(data, labels, n_clusters):
    assert int(n_clusters) == K
    assert data.shape == (B, D, HH, WW)
    x = np.asarray(data, dtype=np.float32).reshape(B, D, N)
    lab = np.asarray(labels).reshape(B, N)
    in_maps = [prep_core_inputs(x[c], lab[c]) for c in range(NCORES)]
    res = run_device(in_maps, trace=False)
    stats = [r["out_stats"] for r in res.results]
    vparts = [r["out_var"] for r in res.results]
    return finish_host(stats, vparts)
